# revision 7
# baseline (speedup 1.0000x reference)
"""CenterLoss kernel for Trainium2 (8 NeuronCores, Bass/Tile).

Strategy
--------
Shard samples across the 8 cores BY CLASS (all samples of a class land on one
core), with each core's samples laid out sorted-by-class into R "bins" of 512
slots such that no class run crosses a bin boundary.  Then on device:

  * center rows are fetched with a transposed dma_gather (bf16, feat-major)
  * per-sample reductions (||xs||^2, xs.c, ||c||^2) are ones-vector matmuls on
    the tensor engine, landing scan-ready [R, 512] tiles in PSUM
  * counts and per-class sums come from segmented prefix scans
    (tensor_tensor_scan) over the sorted layout -- no histogram table, no
    scatter, no count gather:
        sum(dist/cnt) == sum over runs of (run_sum(dist) / run_len)
  * the per-core partial is one scalar; the host adds the 8 partials.

All numerics (normalize, distances, counts, division, reduction) run on
device; the host only permutes/casts inputs (sharding) and sums 8 scalars.
"""

import math
import sys

import numpy as np

sys.path.insert(0, "/opt/trn_rl_repo")

import ml_dtypes  # noqa: E402

import concourse.bass as bass  # noqa: E402
import concourse.tile as tile  # noqa: E402
from concourse import bacc, mybir  # noqa: E402

BF16 = mybir.dt.bfloat16
F32 = mybir.dt.float32
I16 = mybir.dt.int16
NP_BF16 = ml_dtypes.bfloat16

NCORES = 8
P = 128          # SBUF partitions
TBIN = 512       # slots per bin (= one PSUM bank row of f32)
CHROWS = 8       # bins processed per chunk (8 * 512 = 4096 samples)
EPS = 1e-12      # F.normalize eps (matches reference)
PAD_LABEL = -5.0
ALU = mybir.AluOpType


# --------------------------------------------------------------------------
# host-side sharding / layout
# --------------------------------------------------------------------------

def _plan_shards(ys):
    """Split samples into 8 class-contiguous shards and pack each shard's
    class-runs into bins of TBIN slots (runs never cross a bin boundary)."""
    B = ys.shape[0]
    order = np.argsort(ys, kind="stable")
    ys_s = ys[order]
    change = np.flatnonzero(ys_s[1:] != ys_s[:-1]) + 1
    run_starts = np.concatenate([[0], change]).astype(np.int64)
    run_ends = np.concatenate([change, [B]]).astype(np.int64)
    nruns = len(run_starts)

    targets = [(k * B) // NCORES for k in range(1, NCORES)]
    cuts = [0] + [int(c) for c in np.searchsorted(run_ends, targets, side="left")] + [nruns]

    shards = []
    max_bins = 0
    max_cls = 0
    for k in range(NCORES):
        r0, r1 = cuts[k], cuts[k + 1]
        assert r1 > r0, "empty shard"
        # bin-pack runs sequentially
        bins = []          # list of list[(start, end)] into the sorted order
        used = TBIN + 1
        for r in range(r0, r1):
            s, e = int(run_starts[r]), int(run_ends[r])
            L = e - s
            if L > TBIN:
                raise ValueError(f"class run of length {L} exceeds bin size {TBIN}")
            if used + L > TBIN:
                bins.append([])
                used = 0
            bins[-1].append((s, e))
            used += L
        class_lo = int(ys_s[run_starts[r0]])
        class_hi = int(ys_s[run_starts[r1 - 1]])
        shards.append(dict(bins=bins, class_lo=class_lo))
        max_bins = max(max_bins, len(bins))
        max_cls = max(max_cls, class_hi - class_lo + 1)
    R = max_bins
    CLSP = max_cls
    return order, shards, R, CLSP


def _build_core_inputs(xs, ys, center, order, shard, R, CLSP):
    NS = R * TBIN
    class_lo = shard["class_lo"]

    slot_sample = np.full(NS, -1, dtype=np.int64)
    for r, runs in enumerate(shard["bins"]):
        off = r * TBIN
        for (s, e) in runs:
            L = e - s
            slot_sample[off:off + L] = order[s:e]
            off += L

    valid = slot_sample >= 0
    samp = np.where(valid, slot_sample, 0)

    xs_slot = xs[samp]
    xs_slot[~valid] = 0.0
    xsT = np.ascontiguousarray(xs_slot.T).astype(NP_BF16)       # [128, NS]

    loc = (ys[samp] - class_lo).astype(np.int64)
    loc[~valid] = 0
    assert loc.min() >= 0 and loc.max() < CLSP
    cidx_flat = loc.astype(np.int16)
    w = cidx_flat.reshape(NS // 16, 16).T                        # [16, NS/16]
    cidx = np.ascontiguousarray(np.tile(w, (8, 1)))              # [128, NS/16]

    labv = np.where(valid, loc.astype(np.float32), np.float32(PAD_LABEL))
    lab = np.empty((R, TBIN + 2), dtype=np.float32)
    lab[:, 1:-1] = labv.reshape(R, TBIN)
    lab[:, 0] = -1.0
    lab[:, -1] = -2.0

    ctab = np.zeros((CLSP, P), dtype=NP_BF16)
    n = min(CLSP, center.shape[0] - class_lo)
    ctab[:n] = center[class_lo:class_lo + n].astype(NP_BF16)

    return {"xsT": xsT, "cidx": cidx, "labels": lab, "ctab": ctab}


# --------------------------------------------------------------------------
# device program
# --------------------------------------------------------------------------

def build_program(R, CLSP, debug=False):
    NS = R * TBIN
    nc = bacc.Bacc("TRN2", target_bir_lowering=False, debug=debug)

    xsT = nc.dram_tensor("xsT", [P, NS], BF16, kind="ExternalInput")
    ctab = nc.dram_tensor("ctab", [CLSP, P], BF16, kind="ExternalInput")
    cidx = nc.dram_tensor("cidx", [P, NS // 16], I16, kind="ExternalInput")
    labels = nc.dram_tensor("labels", [R, TBIN + 2], F32, kind="ExternalInput")
    out = nc.dram_tensor("out", [1, 1], F32, kind="ExternalOutput")

    CHUNK = CHROWS * TBIN
    nchunks = math.ceil(R / CHROWS)

    with tile.TileContext(nc) as tc:
        with (
            tc.tile_pool(name="io", bufs=3) as io,
            tc.tile_pool(name="prod", bufs=2) as prod,
            tc.tile_pool(name="small", bufs=1) as small,
            tc.tile_pool(name="scanp", bufs=1) as scanp,
            tc.tile_pool(name="psum", bufs=1, space="PSUM") as psum,
        ):
            # basis[:, k, :] is a [128, 32] lhsT whose column k is all-ones:
            # matmul(lhsT=basis[:, k, :], rhs=X) -> row k of a 32-row PSUM
            # group gets colsum(X), other rows get 0 (accumulate-safe).
            basis = small.tile([P, 32, 32], BF16)
            nc.vector.memset(basis[:], 0.0)
            for m in range(32):
                nc.vector.memset(basis[:, m, m:m + 1], 1.0)
            ones_f = small.tile([P, 1], F32)
            nc.vector.memset(ones_f[:], 1.0)
            ones_row = small.tile([R, TBIN], F32)
            nc.vector.memset(ones_row[:], 1.0)

            cidx_sb = small.tile([P, NS // 16], I16)
            nc.sync.dma_start(out=cidx_sb[:], in_=cidx[:, :])
            lab_sb = small.tile([R, TBIN + 2], F32)
            nc.sync.dma_start(out=lab_sb[:], in_=labels[:, :])

            ps_n2 = psum.tile([P, TBIN], F32)
            ps_dot = psum.tile([P, TBIN], F32)
            ps_csq = psum.tile([P, TBIN], F32)

            for g in range(nchunks):
                r0 = g * CHROWS
                nr = min(R, r0 + CHROWS) - r0
                n = nr * TBIN
                xs_t = io.tile([P, CHUNK], BF16, tag="xs")
                c_t = io.tile([P, 1, CHUNK], BF16, tag="ct")
                nc.sync.dma_start(
                    out=xs_t[:, :n], in_=xsT[:, r0 * TBIN:r0 * TBIN + n]
                )
                nc.gpsimd.dma_gather(
                    out_ap=c_t[:, :, :n],
                    in_ap=ctab[:, :],
                    idxs_ap=cidx_sb[:, (r0 * TBIN) // 16:(r0 * TBIN + n) // 16],
                    num_idxs=n,
                    num_idxs_reg=n,
                    elem_size=P,
                    transpose=True,
                    single_packet=False,
                )
                c2d = c_t[:, 0, :]
                xsq = prod.tile([P, CHUNK], BF16, tag="xsq")
                xc = prod.tile([P, CHUNK], BF16, tag="xc")
                csq = prod.tile([P, CHUNK], BF16, tag="csq")
                nc.vector.tensor_tensor(
                    out=xsq[:, :n], in0=xs_t[:, :n], in1=xs_t[:, :n], op=ALU.mult
                )
                nc.vector.tensor_tensor(
                    out=xc[:, :n], in0=xs_t[:, :n], in1=c2d[:, :n], op=ALU.mult
                )
                nc.vector.tensor_tensor(
                    out=csq[:, :n], in0=c2d[:, :n], in1=c2d[:, :n], op=ALU.mult
                )
                for j in range(nr):
                    r = r0 + j
                    sl = slice(j * TBIN, (j + 1) * TBIN)
                    grp = (r // 32) * 32
                    first = r % 32 == 0
                    last = (r == R - 1) or (r % 32 == 31)
                    for ps, rhs_ in (
                        (ps_n2, xsq), (ps_dot, xc), (ps_csq, csq)
                    ):
                        nc.tensor.matmul(
                            out=ps[grp:grp + 32, :],
                            lhsT=basis[:, r % 32, :],
                            rhs=rhs_[:, sl],
                            start=first, stop=last,
                            tile_position=(0, grp),
                        )

            # ---- phase 2: per-sample scalars on [R, TBIN] ----
            def t(name):
                return scanp.tile([R, TBIN], F32, name=name, tag=name)

            n2 = t("n2")
            dot = t("dot")
            csqs = t("csqs")
            nc.scalar.copy(out=n2[:], in_=ps_n2[:R, :])
            nc.scalar.copy(out=dot[:], in_=ps_dot[:R, :])
            nc.scalar.copy(out=csqs[:], in_=ps_csq[:R, :])

            # norm = sqrt(n2), Newton-refined:  y' = 0.5*(y + n2/y)
            y = t("y")
            nc.scalar.sqrt(out=y[:], in_=n2[:])
            nc.vector.tensor_scalar(
                out=y[:], in0=y[:], scalar1=1e-20, scalar2=None, op0=ALU.max
            )
            ry = t("ry")
            nc.vector.reciprocal(out=ry[:], in_=y[:])
            tt = t("tt")
            nc.vector.tensor_tensor(out=tt[:], in0=n2[:], in1=ry[:], op=ALU.mult)
            norm = t("norm")
            nc.vector.tensor_tensor(out=norm[:], in0=tt[:], in1=y[:], op=ALU.add)
            nc.vector.tensor_scalar(
                out=norm[:], in0=norm[:], scalar1=0.5, scalar2=None, op0=ALU.mult
            )
            # r = 1 / max(norm, EPS)
            nc.vector.tensor_scalar(
                out=norm[:], in0=norm[:], scalar1=EPS, scalar2=None, op0=ALU.max
            )
            rs = t("rs")
            nc.vector.reciprocal(out=rs[:], in_=norm[:])

            # dist^2 = r^2*n2 - 2*r*dot + csq
            xnc = t("xnc")
            nc.vector.tensor_tensor(out=xnc[:], in0=dot[:], in1=rs[:], op=ALU.mult)
            r2 = t("r2")
            nc.vector.tensor_tensor(out=r2[:], in0=rs[:], in1=rs[:], op=ALU.mult)
            t3 = t("t3")
            nc.vector.tensor_tensor(out=t3[:], in0=r2[:], in1=n2[:], op=ALU.mult)
            u = t("u")
            nc.vector.scalar_tensor_tensor(
                out=u[:], in0=xnc[:], scalar=-2.0, in1=csqs[:],
                op0=ALU.mult, op1=ALU.add,
            )
            d2 = t("d2")
            nc.vector.tensor_tensor(out=d2[:], in0=t3[:], in1=u[:], op=ALU.add)
            nc.vector.tensor_scalar(
                out=d2[:], in0=d2[:], scalar1=0.0, scalar2=None, op0=ALU.max
            )

            # dist = sqrt(d2), Newton-refined
            yd = t("yd")
            nc.scalar.sqrt(out=yd[:], in_=d2[:])
            nc.vector.tensor_scalar(
                out=yd[:], in0=yd[:], scalar1=1e-20, scalar2=None, op0=ALU.max
            )
            ryd = t("ryd")
            nc.vector.reciprocal(out=ryd[:], in_=yd[:])
            td = t("td")
            nc.vector.tensor_tensor(out=td[:], in0=d2[:], in1=ryd[:], op=ALU.mult)
            dist = t("dist")
            nc.vector.tensor_tensor(out=dist[:], in0=td[:], in1=yd[:], op=ALU.add)
            nc.vector.tensor_scalar(
                out=dist[:], in0=dist[:], scalar1=0.5, scalar2=None, op0=ALU.mult
            )

            # ---- run structure from halo'd labels ----
            lab_c = lab_sb[:, 1:TBIN + 1]
            cont = t("cont")
            nc.vector.tensor_tensor(
                out=cont[:], in0=lab_c, in1=lab_sb[:, 0:TBIN], op=ALU.is_equal
            )
            endf = t("endf")
            nc.vector.tensor_tensor(
                out=endf[:], in0=lab_c, in1=lab_sb[:, 2:TBIN + 2], op=ALU.not_equal
            )
            msk = t("msk")
            nc.vector.tensor_scalar(
                out=msk[:], in0=lab_c, scalar1=0.0, scalar2=None, op0=ALU.is_ge
            )
            endm = t("endm")
            nc.vector.tensor_tensor(out=endm[:], in0=endf[:], in1=msk[:], op=ALU.mult)

            # segmented scans: S = run-sum(dist), C = run-position
            S = t("S")
            nc.vector.tensor_tensor_scan(
                out=S[:], data0=cont[:], data1=dist[:], initial=0.0,
                op0=ALU.mult, op1=ALU.add,
            )
            C = t("C")
            nc.vector.tensor_tensor_scan(
                out=C[:], data0=cont[:], data1=ones_row[:], initial=0.0,
                op0=ALU.mult, op1=ALU.add,
            )
            iC = t("iC")
            nc.vector.reciprocal(out=iC[:], in_=C[:])
            v = t("v")
            nc.vector.tensor_tensor(out=v[:], in0=S[:], in1=iC[:], op=ALU.mult)
            nc.vector.tensor_tensor(out=v[:], in0=v[:], in1=endm[:], op=ALU.mult)

            # ---- final reduction to a scalar ----
            part = scanp.tile([R, 1], F32)
            nc.vector.tensor_reduce(
                out=part[:], in_=v[:], axis=mybir.AxisListType.X, op=ALU.add
            )
            part128 = small.tile([P, 1], F32)
            nc.vector.memset(part128[:], 0.0)
            nc.vector.tensor_copy(out=part128[:R, :], in_=part[:])
            ps_s = psum.tile([1, 1], F32)
            nc.tensor.matmul(
                out=ps_s[:], lhsT=part128[:], rhs=ones_f[:], start=True, stop=True
            )
            res = small.tile([1, 1], F32)
            nc.vector.tensor_copy(out=res[:], in_=ps_s[:])
            nc.sync.dma_start(out=out[:, :], in_=res[:])

    return nc


# --------------------------------------------------------------------------
# entry point
# --------------------------------------------------------------------------

_PROG_CACHE = {}


def _prepare(xs, ys, center):
    xs = np.asarray(xs, dtype=np.float32)
    ys = np.asarray(ys).astype(np.int64)
    center = np.asarray(center, dtype=np.float32)
    order, shards, R, CLSP = _plan_shards(ys)
    # round bin count up so the chunk loop shape is stable-ish across calls
    R = max(CHROWS, math.ceil(R / 2) * 2)
    CLSP = math.ceil(CLSP / 16) * 16
    in_maps = [
        _build_core_inputs(xs, ys, center, order, sh, R, CLSP) for sh in shards
    ]
    return in_maps, R, CLSP


def run(xs, ys, center, trace=False):
    in_maps, R, CLSP = _prepare(xs, ys, center)
    key = (R, CLSP)
    if key not in _PROG_CACHE:
        nc = build_program(R, CLSP)
        nc.finalize()
        _PROG_CACHE[key] = nc
    nc = _PROG_CACHE[key]

    from concourse.bass_utils import run_bass_kernel_spmd

    res = run_bass_kernel_spmd(
        nc, in_maps, list(range(NCORES)), trace=trace
    )
    total = sum(float(np.asarray(r["out"])[0, 0]) for r in res.results)
    return np.float32(total), res


def kernel(xs, ys, center):
    return run(xs, ys, center)[0]


# revision 8
# speedup vs baseline: 954.0730x; 954.0730x over previous
"""CenterLoss kernel for Trainium2 (8 NeuronCores, Bass/Tile).

Strategy
--------
Shard samples across the 8 cores BY CLASS (all samples of a class land on one
core), with each core's samples laid out sorted-by-class into R "bins" of 512
slots such that no class run crosses a bin boundary.  Then on device:

  * center rows are fetched with a transposed dma_gather (bf16, feat-major)
  * per-sample reductions (||xs||^2, xs.c, ||c||^2) are ones-vector matmuls on
    the tensor engine, landing scan-ready [R, 512] tiles in PSUM
  * counts and per-class sums come from segmented prefix scans
    (tensor_tensor_scan) over the sorted layout -- no histogram table, no
    scatter, no count gather:
        sum(dist/cnt) == sum over runs of (run_sum(dist) / run_len)
  * the per-core partial is one scalar; the host adds the 8 partials.

All numerics (normalize, distances, counts, division, reduction) run on
device; the host only permutes/casts inputs (sharding) and sums 8 scalars.
"""

import math
import sys

import numpy as np

sys.path.insert(0, "/opt/trn_rl_repo")

import ml_dtypes  # noqa: E402

import concourse.bass as bass  # noqa: E402
import concourse.tile as tile  # noqa: E402
from concourse import bacc, mybir  # noqa: E402

BF16 = mybir.dt.bfloat16
F32 = mybir.dt.float32
I16 = mybir.dt.int16
NP_BF16 = ml_dtypes.bfloat16

NCORES = 8
P = 128          # SBUF partitions
TBIN = 512       # slots per bin (= one PSUM bank row of f32)
CHROWS = 8       # bins processed per chunk (8 * 512 = 4096 samples)
EPS = 1e-12      # F.normalize eps (matches reference)
PAD_LABEL = -5.0
ALU = mybir.AluOpType


# --------------------------------------------------------------------------
# host-side sharding / layout
# --------------------------------------------------------------------------

def _plan_shards(ys):
    """Split samples into 8 class-contiguous shards and pack each shard's
    class-runs into bins of TBIN slots (runs never cross a bin boundary)."""
    B = ys.shape[0]
    order = np.argsort(ys, kind="stable")
    ys_s = ys[order]
    change = np.flatnonzero(ys_s[1:] != ys_s[:-1]) + 1
    run_starts = np.concatenate([[0], change]).astype(np.int64)
    run_ends = np.concatenate([change, [B]]).astype(np.int64)
    nruns = len(run_starts)

    targets = [(k * B) // NCORES for k in range(1, NCORES)]
    cuts = [0] + [int(c) for c in np.searchsorted(run_ends, targets, side="left")] + [nruns]

    shards = []
    max_bins = 0
    max_cls = 0
    for k in range(NCORES):
        r0, r1 = cuts[k], cuts[k + 1]
        if r1 <= r0:
            shards.append(dict(bins=[[]], class_lo=0))
            max_bins = max(max_bins, 1)
            max_cls = max(max_cls, 1)
            continue
        # bin-pack runs sequentially
        bins = []          # list of list[(start, end)] into the sorted order
        used = TBIN + 1
        for r in range(r0, r1):
            s, e = int(run_starts[r]), int(run_ends[r])
            L = e - s
            if L > TBIN:
                raise ValueError(f"class run of length {L} exceeds bin size {TBIN}")
            if used + L > TBIN:
                bins.append([])
                used = 0
            bins[-1].append((s, e))
            used += L
        class_lo = int(ys_s[run_starts[r0]])
        class_hi = int(ys_s[run_starts[r1 - 1]])
        shards.append(dict(bins=bins, class_lo=class_lo))
        max_bins = max(max_bins, len(bins))
        max_cls = max(max_cls, class_hi - class_lo + 1)
    R = max_bins
    CLSP = max_cls
    return order, shards, R, CLSP


def _build_core_inputs(xs, ys, center, order, shard, R, CLSP):
    NS = R * TBIN
    class_lo = shard["class_lo"]

    slot_sample = np.full(NS, -1, dtype=np.int64)
    for r, runs in enumerate(shard["bins"]):
        off = r * TBIN
        for (s, e) in runs:
            L = e - s
            slot_sample[off:off + L] = order[s:e]
            off += L

    valid = slot_sample >= 0
    samp = np.where(valid, slot_sample, 0)

    xs_slot = xs[samp]
    xs_slot[~valid] = 0.0
    xsT = np.ascontiguousarray(xs_slot.T).astype(NP_BF16)       # [128, NS]

    loc = (ys[samp] - class_lo).astype(np.int64)
    loc[~valid] = 0
    assert loc.min() >= 0 and loc.max() < CLSP
    cidx_flat = loc.astype(np.int16)
    w = cidx_flat.reshape(NS // 16, 16).T                        # [16, NS/16]
    cidx = np.ascontiguousarray(np.tile(w, (8, 1)))              # [128, NS/16]

    labv = np.where(valid, loc.astype(np.float32), np.float32(PAD_LABEL))
    lab = np.empty((R, TBIN + 2), dtype=np.float32)
    lab[:, 1:-1] = labv.reshape(R, TBIN)
    lab[:, 0] = -1.0
    lab[:, -1] = -2.0

    ctab = np.zeros((CLSP, P), dtype=NP_BF16)
    n = min(CLSP, center.shape[0] - class_lo)
    ctab[:n] = center[class_lo:class_lo + n].astype(NP_BF16)

    return {"xsT": xsT, "cidx": cidx, "labels": lab, "ctab": ctab}


# --------------------------------------------------------------------------
# device program
# --------------------------------------------------------------------------

def build_program(R, CLSP, debug=False):
    NS = R * TBIN
    nc = bacc.Bacc("TRN2", target_bir_lowering=False, debug=debug)

    xsT = nc.dram_tensor("xsT", [P, NS], BF16, kind="ExternalInput")
    ctab = nc.dram_tensor("ctab", [CLSP, P], BF16, kind="ExternalInput")
    cidx = nc.dram_tensor("cidx", [P, NS // 16], I16, kind="ExternalInput")
    labels = nc.dram_tensor("labels", [R, TBIN + 2], F32, kind="ExternalInput")
    out = nc.dram_tensor("out", [1, 1], F32, kind="ExternalOutput")

    CHUNK = CHROWS * TBIN
    nchunks = math.ceil(R / CHROWS)

    with tile.TileContext(nc) as tc:
        with (
            tc.tile_pool(name="io", bufs=3) as io,
            tc.tile_pool(name="prod", bufs=2) as prod,
            tc.tile_pool(name="small", bufs=1) as small,
            tc.tile_pool(name="scanp", bufs=1) as scanp,
            tc.tile_pool(name="psum", bufs=1, space="PSUM") as psum,
        ):
            # basis[:, k, :] is a [128, 32] lhsT whose column k is all-ones:
            # matmul(lhsT=basis[:, k, :], rhs=X) -> row k of a 32-row PSUM
            # group gets colsum(X), other rows get 0 (accumulate-safe).
            basis = small.tile([P, 32, 32], BF16)
            nc.vector.memset(basis[:], 0.0)
            for m in range(32):
                nc.vector.memset(basis[:, m, m:m + 1], 1.0)
            ones_f = small.tile([P, 1], F32)
            nc.vector.memset(ones_f[:], 1.0)
            ones_row = small.tile([R, TBIN], F32)
            nc.vector.memset(ones_row[:], 1.0)

            cidx_sb = small.tile([P, NS // 16], I16)
            nc.sync.dma_start(out=cidx_sb[:], in_=cidx[:, :])
            lab_sb = small.tile([R, TBIN + 2], F32)
            nc.sync.dma_start(out=lab_sb[:], in_=labels[:, :])

            ps_n2 = psum.tile([P, TBIN], F32)
            ps_dot = psum.tile([P, TBIN], F32)
            ps_csq = psum.tile([P, TBIN], F32)

            for g in range(nchunks):
                r0 = g * CHROWS
                nr = min(R, r0 + CHROWS) - r0
                n = nr * TBIN
                xs_t = io.tile([P, CHUNK], BF16, tag="xs")
                c_t = io.tile([P, 1, CHUNK], BF16, tag="ct")
                nc.sync.dma_start(
                    out=xs_t[:, :n], in_=xsT[:, r0 * TBIN:r0 * TBIN + n]
                )
                nc.gpsimd.dma_gather(
                    out_ap=c_t[:, :, :n],
                    in_ap=ctab[:, :],
                    idxs_ap=cidx_sb[:, (r0 * TBIN) // 16:(r0 * TBIN + n) // 16],
                    num_idxs=n,
                    num_idxs_reg=n,
                    elem_size=P,
                    transpose=True,
                    single_packet=False,
                )
                c2d = c_t[:, 0, :]
                xsq = prod.tile([P, CHUNK], BF16, tag="xsq")
                xc = prod.tile([P, CHUNK], BF16, tag="xc")
                csq = prod.tile([P, CHUNK], BF16, tag="csq")
                nc.vector.tensor_tensor(
                    out=xsq[:, :n], in0=xs_t[:, :n], in1=xs_t[:, :n], op=ALU.mult
                )
                nc.vector.tensor_tensor(
                    out=xc[:, :n], in0=xs_t[:, :n], in1=c2d[:, :n], op=ALU.mult
                )
                nc.vector.tensor_tensor(
                    out=csq[:, :n], in0=c2d[:, :n], in1=c2d[:, :n], op=ALU.mult
                )
                for j in range(nr):
                    r = r0 + j
                    sl = slice(j * TBIN, (j + 1) * TBIN)
                    grp = (r // 32) * 32
                    first = r % 32 == 0
                    last = (r == R - 1) or (r % 32 == 31)
                    for ps, rhs_ in (
                        (ps_n2, xsq), (ps_dot, xc), (ps_csq, csq)
                    ):
                        nc.tensor.matmul(
                            out=ps[grp:grp + 32, :],
                            lhsT=basis[:, r % 32, :],
                            rhs=rhs_[:, sl],
                            start=first, stop=last,
                            tile_position=(0, grp),
                        )

            # ---- phase 2: per-sample scalars on [R, TBIN] ----
            def t(name):
                return scanp.tile([R, TBIN], F32, name=name, tag=name)

            n2 = t("n2")
            dot = t("dot")
            csqs = t("csqs")
            nc.scalar.copy(out=n2[:], in_=ps_n2[:R, :])
            nc.scalar.copy(out=dot[:], in_=ps_dot[:R, :])
            nc.scalar.copy(out=csqs[:], in_=ps_csq[:R, :])

            # norm = sqrt(n2), Newton-refined:  y' = 0.5*(y + n2/y)
            y = t("y")
            nc.scalar.sqrt(out=y[:], in_=n2[:])
            nc.vector.tensor_scalar(
                out=y[:], in0=y[:], scalar1=1e-20, scalar2=None, op0=ALU.max
            )
            ry = t("ry")
            nc.vector.reciprocal(out=ry[:], in_=y[:])
            tt = t("tt")
            nc.vector.tensor_tensor(out=tt[:], in0=n2[:], in1=ry[:], op=ALU.mult)
            norm = t("norm")
            nc.vector.tensor_tensor(out=norm[:], in0=tt[:], in1=y[:], op=ALU.add)
            nc.vector.tensor_scalar(
                out=norm[:], in0=norm[:], scalar1=0.5, scalar2=None, op0=ALU.mult
            )
            # r = 1 / max(norm, EPS)
            nc.vector.tensor_scalar(
                out=norm[:], in0=norm[:], scalar1=EPS, scalar2=None, op0=ALU.max
            )
            rs = t("rs")
            nc.vector.reciprocal(out=rs[:], in_=norm[:])

            # dist^2 = r^2*n2 - 2*r*dot + csq
            xnc = t("xnc")
            nc.vector.tensor_tensor(out=xnc[:], in0=dot[:], in1=rs[:], op=ALU.mult)
            r2 = t("r2")
            nc.vector.tensor_tensor(out=r2[:], in0=rs[:], in1=rs[:], op=ALU.mult)
            t3 = t("t3")
            nc.vector.tensor_tensor(out=t3[:], in0=r2[:], in1=n2[:], op=ALU.mult)
            u = t("u")
            nc.vector.scalar_tensor_tensor(
                out=u[:], in0=xnc[:], scalar=-2.0, in1=csqs[:],
                op0=ALU.mult, op1=ALU.add,
            )
            d2 = t("d2")
            nc.vector.tensor_tensor(out=d2[:], in0=t3[:], in1=u[:], op=ALU.add)
            nc.vector.tensor_scalar(
                out=d2[:], in0=d2[:], scalar1=0.0, scalar2=None, op0=ALU.max
            )

            # dist = sqrt(d2), Newton-refined
            yd = t("yd")
            nc.scalar.sqrt(out=yd[:], in_=d2[:])
            nc.vector.tensor_scalar(
                out=yd[:], in0=yd[:], scalar1=1e-20, scalar2=None, op0=ALU.max
            )
            ryd = t("ryd")
            nc.vector.reciprocal(out=ryd[:], in_=yd[:])
            td = t("td")
            nc.vector.tensor_tensor(out=td[:], in0=d2[:], in1=ryd[:], op=ALU.mult)
            dist = t("dist")
            nc.vector.tensor_tensor(out=dist[:], in0=td[:], in1=yd[:], op=ALU.add)
            nc.vector.tensor_scalar(
                out=dist[:], in0=dist[:], scalar1=0.5, scalar2=None, op0=ALU.mult
            )

            # ---- run structure from halo'd labels ----
            lab_c = lab_sb[:, 1:TBIN + 1]
            cont = t("cont")
            nc.vector.tensor_tensor(
                out=cont[:], in0=lab_c, in1=lab_sb[:, 0:TBIN], op=ALU.is_equal
            )
            endf = t("endf")
            nc.vector.tensor_tensor(
                out=endf[:], in0=lab_c, in1=lab_sb[:, 2:TBIN + 2], op=ALU.not_equal
            )
            msk = t("msk")
            nc.vector.tensor_scalar(
                out=msk[:], in0=lab_c, scalar1=0.0, scalar2=None, op0=ALU.is_ge
            )
            endm = t("endm")
            nc.vector.tensor_tensor(out=endm[:], in0=endf[:], in1=msk[:], op=ALU.mult)

            # segmented scans: S = run-sum(dist), C = run-position
            S = t("S")
            nc.vector.tensor_tensor_scan(
                out=S[:], data0=cont[:], data1=dist[:], initial=0.0,
                op0=ALU.mult, op1=ALU.add,
            )
            C = t("C")
            nc.vector.tensor_tensor_scan(
                out=C[:], data0=cont[:], data1=ones_row[:], initial=0.0,
                op0=ALU.mult, op1=ALU.add,
            )
            iC = t("iC")
            nc.vector.reciprocal(out=iC[:], in_=C[:])
            v = t("v")
            nc.vector.tensor_tensor(out=v[:], in0=S[:], in1=iC[:], op=ALU.mult)
            nc.vector.tensor_tensor(out=v[:], in0=v[:], in1=endm[:], op=ALU.mult)

            # ---- final reduction to a scalar ----
            part = scanp.tile([R, 1], F32)
            nc.vector.tensor_reduce(
                out=part[:], in_=v[:], axis=mybir.AxisListType.X, op=ALU.add
            )
            part128 = small.tile([P, 1], F32)
            nc.vector.memset(part128[:], 0.0)
            nc.vector.tensor_copy(out=part128[:R, :], in_=part[:])
            ps_s = psum.tile([1, 1], F32)
            nc.tensor.matmul(
                out=ps_s[:], lhsT=part128[:], rhs=ones_f[:], start=True, stop=True
            )
            res = small.tile([1, 1], F32)
            nc.vector.tensor_copy(out=res[:], in_=ps_s[:])
            nc.sync.dma_start(out=out[:, :], in_=res[:])

    return nc


# --------------------------------------------------------------------------
# entry point
# --------------------------------------------------------------------------

_PROG_CACHE = {}


def _prepare(xs, ys, center):
    xs = np.asarray(xs, dtype=np.float32)
    ys = np.asarray(ys).astype(np.int64)
    center = np.asarray(center, dtype=np.float32)
    order, shards, R, CLSP = _plan_shards(ys)
    # round bin count up so the chunk loop shape is stable-ish across calls
    R = max(CHROWS, math.ceil(R / 2) * 2)
    CLSP = math.ceil(CLSP / 16) * 16
    in_maps = [
        _build_core_inputs(xs, ys, center, order, sh, R, CLSP) for sh in shards
    ]
    return in_maps, R, CLSP


def run(xs, ys, center, trace=False):
    in_maps, R, CLSP = _prepare(xs, ys, center)
    key = (R, CLSP)
    if key not in _PROG_CACHE:
        nc = build_program(R, CLSP)
        nc.finalize()
        _PROG_CACHE[key] = nc
    nc = _PROG_CACHE[key]

    from concourse.bass_utils import run_bass_kernel_spmd

    res = run_bass_kernel_spmd(
        nc, in_maps, list(range(NCORES)), trace=trace
    )
    total = sum(float(np.asarray(r["out"])[0, 0]) for r in res.results)
    return np.float32(total), res


def kernel(xs, ys, center):
    return run(xs, ys, center)[0]
